# revision 3
# baseline (speedup 1.0000x reference)
"""CutsSelector GNN message-passing kernel for 8 Trainium2 NeuronCores.

Strategy (destination-sharded, no collectives):
  - Edges are sorted by dst on the host; core c owns nodes [c*6250, (c+1)*6250)
    and all edges pointing into that range.
  - Linearity of the message MLP:
        segsum(concat(x_dst, x_src, e) @ g_W + g_b, dst)
      = cnt*(x@gW1 + g_b) + segsum(x[src])@gW2 + segsum(e)@gW3
    so only the raw per-edge features (x[src] rows, edge_attr rows) need the
    segmented reduce; all matmuls happen at node granularity.
  - x[src] rows are fetched with the SWDGE indexed gather (dma_gather) from
    bf16 x banks padded to 256B rows.  int16 index limit -> two 25000-row
    banks.  Windows are batched into GROUPS so each (group, bank) is ONE
    dma_gather call (<= 16368 idxs, the Q7 int32-scratch limit); this cuts
    per-call GPSIMD descriptor-gen overhead ~7x vs per-window calls.
  - Within each (window, bank) edges are sorted by src so the gathered rows
    hit DRAM in ascending address order (page locality).
  - Segmented reduce: per 128-edge tile a one-hot M[e, j] = (dst_rel[e] == j)
    is built ON DEVICE by the idle DVE (tensor_scalar is_equal of a constant
    iota row against the per-slot dst_rel), output fp8; ONE TensorE matmul
    per tile accumulates [segsum_x | segsum_attr] into PSUM.  This removes
    the 26MB host-baked one-hot DMA of the previous version.
  - Node phase per window: 1/cnt scaling fused into the ACT PSUM->SBUF copy,
    PE transpose to feature-major, then g/f/cls MLPs (bf16 in, f32 psum).
  - bf16 rounding can flip the y = probs > 0.5 threshold for nodes whose
    probs sit within ~1e-3 of 0.5; those few nodes are recomputed exactly on
    the host afterwards.
"""
import os
import sys

sys.path.insert(0, "/opt/trn_rl_repo")
os.environ.setdefault("BASS_PERFETTO_PROFILE_ALL_CORES", "1")

import numpy as np

N_NODES = 50000
N_EDGES = 1_600_000
C = 64
D = 16
N_CORES = 8
NPC = N_NODES // N_CORES            # 6250 nodes per core
WPC = (NPC + 127) // 128            # 49 windows per core
NPAD = WPC * 128                    # 6272 padded nodes per core
BANK = 25000                        # gather-bank split (int16 index limit)
GROW = 128                          # gather source row length (bf16 -> 256B)
RHS = C + D                         # 80 columns fed to the reduce matmul
TCAP = 96                           # max tiles per (group, bank) gather call


def _bf16(a):
    import ml_dtypes
    return np.asarray(a, np.float32).astype(ml_dtypes.bfloat16)


def _host_prep(x, src, dst, edge_attr, g_W, g_b, f_W, f_b, cls_W):
    """Sort/shard/pack everything the device program needs."""
    order = np.argsort(dst, kind="stable")
    s_src = src[order]
    s_dst = dst[order]
    s_attr = edge_attr[order]

    core_lo = np.searchsorted(s_dst, np.arange(N_CORES) * NPC)
    core_hi = np.searchsorted(s_dst, (np.arange(N_CORES) + 1) * NPC)

    n0 = np.zeros((N_CORES, WPC), np.int64)
    n1 = np.zeros((N_CORES, WPC), np.int64)
    segs = []
    for c in range(N_CORES):
        sl = slice(core_lo[c], core_hi[c])
        ld = s_dst[sl] - c * NPC
        wb = np.searchsorted(ld, np.arange(WPC + 1) * 128)
        b0 = s_src[sl] < BANK
        segs.append((sl, ld, wb, b0))
        for w in range(WPC):
            seg = slice(wb[w], wb[w + 1])
            n0[c, w] = int(b0[seg].sum())
            n1[c, w] = int(seg.stop - seg.start) - n0[c, w]

    # SPMD-identical tile counts: max over cores, rounded up to whole tiles
    T0 = np.maximum(1, -(-n0.max(axis=0) // 128)).astype(np.int64)
    T1 = np.maximum(1, -(-n1.max(axis=0) // 128)).astype(np.int64)

    # Window groups: one dma_gather call per (group, bank); cap per-call tiles
    groups = []  # list of lists of window ids
    cur, a0, a1 = [], 0, 0
    for w in range(WPC):
        if cur and (a0 + T0[w] > TCAP or a1 + T1[w] > TCAP):
            groups.append(cur)
            cur, a0, a1 = [], 0, 0
        cur.append(w)
        a0 += int(T0[w])
        a1 += int(T1[w])
    if cur:
        groups.append(cur)

    # Slot layout: per group: [bank0 tiles of all windows | bank1 tiles]
    # gmeta: per group: (gtile0, Tg, [(w, off0, t0, off1, t1), ...], b0_tiles)
    gmeta = []
    gtile = 0
    for ws in groups:
        b0_tiles = int(sum(int(T0[w]) for w in ws))
        b1_tiles = int(sum(int(T1[w]) for w in ws))
        wins = []
        o0, o1 = 0, b0_tiles
        for w in ws:
            wins.append((w, o0, int(T0[w]), o1, int(T1[w])))
            o0 += int(T0[w])
            o1 += int(T1[w])
        gmeta.append((gtile, b0_tiles + b1_tiles, wins, b0_tiles))
        gtile += b0_tiles + b1_tiles
    ntiles = gtile
    nslot = ntiles * 128

    per_core = []
    for c in range(N_CORES):
        sl, ld, wb, b0 = segs[c]
        idx_arr = np.zeros(nslot, np.int16)
        dstrel = np.full(nslot, -1.0, np.float32)
        attr_arr = np.zeros((nslot, D), np.float32)
        csrc = s_src[sl]
        cattr = s_attr[sl]
        for (gtile0, Tg, wins, b0t) in gmeta:
            for (w, off0, t0, off1, t1) in wins:
                seg = slice(wb[w], wb[w + 1])
                m0 = b0[seg]
                wsrc = csrc[seg]
                wld = ld[seg]
                wat = cattr[seg]
                for bsel, off in ((m0, (gtile0 + off0) * 128),
                                  (~m0, (gtile0 + off1) * 128)):
                    ssrc = wsrc[bsel]
                    o = np.argsort(ssrc, kind="stable")  # src-sorted: DRAM locality
                    k = int(bsel.sum())
                    ssrc = ssrc[o]
                    if bsel is not m0:
                        ssrc = ssrc - BANK
                    idx_arr[off:off + k] = ssrc.astype(np.int16)
                    dstrel[off:off + k] = (wld[bsel][o] - 128 * w).astype(np.float32)
                    attr_arr[off:off + k] = wat[bsel][o]

        idxs_p = np.tile(idx_arr.reshape(nslot // 16, 16).T, (8, 1)).copy()
        attr_p = _bf16(attr_arr.reshape(ntiles, 128, D).transpose(1, 0, 2))
        dstrel_p = np.ascontiguousarray(dstrel.reshape(ntiles, 128).T)

        cnt = np.bincount(ld, minlength=NPC).astype(np.float32)
        inv = 1.0 / np.maximum(cnt, 1.0)
        r = (cnt > 0).astype(np.float32)
        inv_pad = np.ones(NPAD, np.float32)
        inv_pad[:NPC] = inv
        inv_p = inv_pad.reshape(WPC, 128).T.copy()

        x_loc = x[c * NPC : (c + 1) * NPC]
        xTr = np.zeros((C + 1, NPAD), np.float32)
        xTr[:C, :NPC] = (x_loc * r[:, None]).T
        xTr[C, :NPC] = r
        xT1 = np.zeros((C + 1, NPAD), np.float32)
        xT1[:C, :NPC] = x_loc.T
        xT1[C, :NPC] = 1.0

        per_core.append(
            dict(idxs=idxs_p, attr=attr_p, dstrel=dstrel_p, inv=inv_p,
                 xTr=_bf16(xTr), xT1=_bf16(xT1))
        )

    xpad = np.zeros((N_NODES, GROW), np.float32)
    xpad[:, :C] = x
    iota = np.ascontiguousarray(np.broadcast_to(np.arange(128, dtype=np.float32), (128, 128)))
    shared = dict(
        xb0=_bf16(xpad[:BANK]),
        xb1=_bf16(xpad[BANK:]),
        ident=_bf16(np.eye(128, dtype=np.float32)),
        iota=iota,
        Wg1b=_bf16(np.concatenate([g_W[:C], g_b[None]], 0)),
        Wg23=_bf16(g_W[C:]),
        Wf1b=_bf16(np.concatenate([f_W[:C], f_b[None]], 0)),
        Wf2=_bf16(f_W[C:]),
        Wcls=_bf16(cls_W),
    )
    return per_core, shared, gmeta, ntiles


def _build(gmeta, ntiles, cls_b):
    from concourse import bacc, tile, library_config
    from concourse import mybir

    f32 = mybir.dt.float32
    bf16 = mybir.dt.bfloat16
    fp8 = mybir.dt.float8e4
    nslot = ntiles * 128
    nc = bacc.Bacc(None, num_swdge_queues=4, dynamic_dma_scratch_size=32768)

    xb0_d = nc.declare_dram_parameter("xb0", [BANK, GROW], bf16, isOutput=False)
    xb1_d = nc.declare_dram_parameter("xb1", [N_NODES - BANK, GROW], bf16, isOutput=False)
    idxs_d = nc.declare_dram_parameter("idxs", [128, nslot // 16], mybir.dt.int16, isOutput=False)
    attr_d = nc.declare_dram_parameter("attr", [128, ntiles, D], bf16, isOutput=False)
    dstrel_d = nc.declare_dram_parameter("dstrel", [128, ntiles], f32, isOutput=False)
    ident_d = nc.declare_dram_parameter("ident", [128, 128], bf16, isOutput=False)
    iota_d = nc.declare_dram_parameter("iota", [128, 128], f32, isOutput=False)
    inv_d = nc.declare_dram_parameter("inv", [128, WPC], f32, isOutput=False)
    xTr_d = nc.declare_dram_parameter("xTr", [C + 1, NPAD], bf16, isOutput=False)
    xT1_d = nc.declare_dram_parameter("xT1", [C + 1, NPAD], bf16, isOutput=False)
    Wg1b_d = nc.declare_dram_parameter("Wg1b", [C + 1, C], bf16, isOutput=False)
    Wg23_d = nc.declare_dram_parameter("Wg23", [C + D, C], bf16, isOutput=False)
    Wf1b_d = nc.declare_dram_parameter("Wf1b", [C + 1, C], bf16, isOutput=False)
    Wf2_d = nc.declare_dram_parameter("Wf2", [C, C], bf16, isOutput=False)
    Wcls_d = nc.declare_dram_parameter("Wcls", [C, 1], bf16, isOutput=False)
    out_d = nc.declare_dram_parameter("out", [NPC], f32, isOutput=True)

    TGMAX = max(Tg for (_, Tg, _, _) in gmeta)
    glim = int(os.environ.get("KERNEL_GLIM", len(gmeta)))

    with tile.TileContext(nc) as tc:
        with (
            tc.tile_pool(name="const", bufs=1) as constp,
            tc.tile_pool(name="gx", bufs=2) as gxp,
            tc.tile_pool(name="attr", bufs=2) as attrp,
            tc.tile_pool(name="m", bufs=4) as mp,
            tc.tile_pool(name="nodesb", bufs=2) as nsb,
            tc.tile_pool(name="pacc", bufs=2, space="PSUM") as pacc,
            tc.tile_pool(name="pt", bufs=1, space="PSUM") as ptp,
            tc.tile_pool(name="pn", bufs=1, space="PSUM") as pnp,
        ):
            nc.gpsimd.load_library(library_config.mlp)

            idxs = constp.tile([128, nslot // 16], mybir.dt.int16)
            ident = constp.tile([128, 128], bf16)
            iota = constp.tile([128, 128], f32)
            dstrel = constp.tile([128, ntiles], f32)
            inv = constp.tile([128, WPC], f32)
            xTr = constp.tile([C + 1, NPAD], bf16)
            xT1 = constp.tile([C + 1, NPAD], bf16)
            Wg1b = constp.tile([C + 1, C], bf16)
            Wg23 = constp.tile([C + D, C], bf16)
            Wf1b = constp.tile([C + 1, C], bf16)
            Wf2 = constp.tile([C, C], bf16)
            Wcls = constp.tile([C, 1], bf16)
            probs = constp.tile([1, NPAD], f32)

            nc.sync.dma_start(idxs[:], idxs_d[:])
            nc.sync.dma_start(ident[:], ident_d[:])
            nc.sync.dma_start(iota[:], iota_d[:])
            nc.sync.dma_start(dstrel[:], dstrel_d[:])
            nc.sync.dma_start(inv[:], inv_d[:])
            nc.sync.dma_start(xTr[:], xTr_d[:])
            nc.sync.dma_start(xT1[:], xT1_d[:])
            nc.sync.dma_start(Wg1b[:], Wg1b_d[:])
            nc.sync.dma_start(Wg23[:], Wg23_d[:])
            nc.sync.dma_start(Wf1b[:], Wf1b_d[:])
            nc.sync.dma_start(Wf2[:], Wf2_d[:])
            nc.sync.dma_start(Wcls[:], Wcls_d[:])

            qrr = [0]
            for (gtile0, Tg, wins, b0t) in gmeta[:glim]:
                gx = gxp.tile([128, TGMAX, GROW], bf16, tag="gx")
                at = attrp.tile([128, TGMAX, D], bf16, tag="attr")
                nc.sync.dma_start(at[:, 0:Tg, :], attr_d[:, gtile0:gtile0 + Tg, :])

                # one gather call per (group, bank)
                for xb, lo, n in ((xb0_d, 0, b0t * 128),
                                  (xb1_d, b0t * 128, (Tg - b0t) * 128)):
                    s0 = gtile0 * 128 + lo
                    nc.gpsimd.dma_gather(
                        gx[:, lo // 128 : (lo + n) // 128, :],
                        xb[:],
                        idxs[:, s0 // 16 : (s0 + n) // 16],
                        n, n, GROW, elem_step=GROW,
                        single_packet=False,
                        queue_num=qrr[0] % 4,
                    )
                    qrr[0] += 1

                # edge_attr into the padding columns of the gathered tiles
                nc.scalar.activation(gx[:, 0:Tg, C : C + D], at[:, 0:Tg, :],
                                     mybir.ActivationFunctionType.Copy)

                for (w, off0, t0, off1, t1) in wins:
                    acc = pacc.tile([128, RHS], f32, tag="acc")
                    tiles = [off0 + i for i in range(t0)] + \
                            [off1 + i for i in range(t1)]
                    for k, t in enumerate(tiles):
                        gt = gtile0 + t
                        mw = mp.tile([128, 128], fp8, tag="m")
                        nc.vector.tensor_scalar(
                            mw[:], iota[:], dstrel[:, gt:gt + 1], None,
                            op0=mybir.AluOpType.is_equal)
                        nc.tensor.matmul(acc[:], mw[:], gx[:, t, 0:RHS],
                                         start=(k == 0), stop=(k == len(tiles) - 1))

                    # PSUM -> SBUF with 1/cnt scaling fused into the ACT copy
                    s = nsb.tile([128, RHS], bf16, tag="s")
                    nc.scalar.activation(s[:], acc[:],
                                         mybir.ActivationFunctionType.Copy,
                                         scale=inv[:, w : w + 1])

                    # transpose to feature-major
                    pt1 = ptp.tile([RHS, 128], bf16, tag="pt1")
                    nc.tensor.transpose(pt1[:], s[:], ident[:])
                    sT = nsb.tile([RHS, 128], bf16, tag="sT")
                    nc.scalar.activation(sT[:], pt1[:],
                                         mybir.ActivationFunctionType.Copy)

                    cols = slice(w * 128, (w + 1) * 128)
                    pag = pnp.tile([C, 128], f32, tag="pag")
                    nc.tensor.matmul(pag[:], Wg1b[:], xTr[:, cols], start=True, stop=False)
                    nc.tensor.matmul(pag[:], Wg23[:], sT[:], start=False, stop=True)
                    aggrT = nsb.tile([C, 128], bf16, tag="aggrT")
                    nc.scalar.activation(aggrT[:], pag[:],
                                         mybir.ActivationFunctionType.Copy)

                    ph = pnp.tile([C, 128], f32, tag="ph")
                    nc.tensor.matmul(ph[:], Wf1b[:], xT1[:, cols], start=True, stop=False)
                    nc.tensor.matmul(ph[:], Wf2[:], aggrT[:], start=False, stop=True)
                    hT = nsb.tile([C, 128], bf16, tag="hT")
                    nc.scalar.activation(hT[:], ph[:],
                                         mybir.ActivationFunctionType.Copy)

                    pl = pnp.tile([1, 128], f32, tag="pl")
                    nc.tensor.matmul(pl[:], Wcls[:], hT[:], start=True, stop=True)
                    nc.scalar.activation(probs[0:1, cols], pl[:],
                                         mybir.ActivationFunctionType.Sigmoid,
                                         bias=float(cls_b))

            nc.sync.dma_start(out_d[0:NPC], probs[0:1, 0:NPC])

    nc.compile()
    return nc


def _exact_patch(probs, sel, x, src, dst, edge_attr, g_W, g_b, f_W, f_b,
                 cls_W, cls_b):
    """Recompute probs exactly (f64) for the selected nodes."""
    if not sel.any():
        return probs
    nodes = np.nonzero(sel)[0]
    order = np.argsort(dst, kind="stable")
    s_src = src[order]
    s_dst = dst[order]
    s_attr = edge_attr[order].astype(np.float64)
    lo = np.searchsorted(s_dst, nodes)
    hi = np.searchsorted(s_dst, nodes + 1)
    x64 = x.astype(np.float64)
    gW = g_W.astype(np.float64)
    fW = f_W.astype(np.float64)
    for i, node in enumerate(nodes):
        e = slice(lo[i], hi[i])
        cntv = hi[i] - lo[i]
        if cntv > 0:
            z = np.concatenate([
                np.broadcast_to(x64[node], (cntv, C)),
                x64[s_src[e]],
                s_attr[e],
            ], axis=1)
            aggr = (z @ gW + g_b).sum(0) / cntv
        else:
            aggr = np.zeros(C)
        h = np.concatenate([x64[node], aggr]) @ fW + f_b
        logit = h @ cls_W.astype(np.float64)[:, 0] + cls_b
        probs[node] = 1.0 / (1.0 + np.exp(-logit))
    return probs


def kernel(x, edge_index, edge_attr, g_W, g_b, f_W, f_b, cls_W, cls_b):
    from concourse.bass_utils import run_bass_kernel_spmd

    x = np.asarray(x, np.float32)
    edge_attr = np.asarray(edge_attr, np.float32)
    src = np.asarray(edge_index[0], np.int64)
    dst = np.asarray(edge_index[1], np.int64)
    g_W = np.asarray(g_W, np.float32)
    g_b = np.asarray(g_b, np.float32)
    f_W = np.asarray(f_W, np.float32)
    f_b = np.asarray(f_b, np.float32)
    cls_W = np.asarray(cls_W, np.float32)
    cls_bv = float(np.asarray(cls_b).reshape(-1)[0])

    per_core, shared, gmeta, ntiles = _host_prep(
        x, src, dst, edge_attr, g_W, g_b, f_W, f_b, cls_W)
    nc = _build(gmeta, ntiles, cls_bv)

    in_maps = [{**shared, **pc} for pc in per_core]
    res = run_bass_kernel_spmd(nc, in_maps, core_ids=list(range(N_CORES)))
    probs = np.concatenate([res.results[c]["out"] for c in range(N_CORES)])
    probs = probs.astype(np.float64)

    # bf16 device math can flip the 0.5 threshold for near-boundary nodes;
    # recompute those exactly.
    sel = np.abs(probs - 0.5) < 2e-3
    probs = _exact_patch(probs, sel, x, src, dst, edge_attr, g_W, g_b,
                         f_W, f_b, cls_W, cls_bv)

    probs = probs.reshape(N_NODES, 1).astype(np.float32)
    y = probs > 0.5
    return (y, probs)


# revision 10
# speedup vs baseline: 1.1269x; 1.1269x over previous
"""CutsSelector GNN message-passing kernel for 8 Trainium2 NeuronCores.

Strategy (destination-sharded, no collectives):
  - Edges are sorted by dst on the host; core c owns nodes [c*6250, (c+1)*6250)
    and all edges pointing into that range.
  - Linearity of the message MLP:
        segsum(concat(x_dst, x_src, e) @ g_W + g_b, dst)
      = cnt*(x@gW1 + g_b) + segsum(x[src])@gW2 + segsum(e)@gW3
    so only the raw per-edge features (x[src] rows, edge_attr rows) need the
    segmented reduce; all matmuls happen at node granularity.
  - x[src] rows are fetched with the SWDGE indexed gather (dma_gather) from
    bf16 x banks padded to 256B rows.  int16 index limit -> two 25000-row
    banks.  Windows are batched into GROUPS so each (group, bank) is ONE
    dma_gather call (<= 16368 idxs, the Q7 int32-scratch limit); this cuts
    per-call GPSIMD descriptor-gen overhead ~7x vs per-window calls.
  - Within each (window, bank) edges are sorted by src so the gathered rows
    hit DRAM in ascending address order (page locality).
  - Segmented reduce: per 128-edge tile a one-hot M[e, j] = (dst_rel[e] == j)
    is built ON DEVICE by the idle DVE (tensor_scalar is_equal of a constant
    iota row against the per-slot dst_rel), output fp8; ONE TensorE matmul
    per tile accumulates [segsum_x | segsum_attr] into PSUM.  This removes
    the 26MB host-baked one-hot DMA of the previous version.
  - Node phase per window: 1/cnt scaling fused into the ACT PSUM->SBUF copy,
    PE transpose to feature-major, then g/f/cls MLPs (bf16 in, f32 psum).
  - bf16 rounding can flip the y = probs > 0.5 threshold for nodes whose
    probs sit within ~1e-3 of 0.5; those few nodes are recomputed exactly on
    the host afterwards.
"""
import os
import sys

sys.path.insert(0, "/opt/trn_rl_repo")
os.environ.setdefault("BASS_PERFETTO_PROFILE_ALL_CORES", "1")

import numpy as np

N_NODES = 50000
N_EDGES = 1_600_000
C = 64
D = 16
N_CORES = 8
NPC = N_NODES // N_CORES            # 6250 nodes per core
WPC = (NPC + 127) // 128            # 49 windows per core
NPAD = WPC * 128                    # 6272 padded nodes per core
BANK = 25000                        # gather-bank split (int16 index limit)
GROW = 128                          # gather source row length (bf16 -> 256B)
RHS = C + D                         # 80 columns fed to the reduce matmul
TCAP = 80                           # max tiles per (group, bank) gather call


def _bf16(a):
    import ml_dtypes
    return np.asarray(a, np.float32).astype(ml_dtypes.bfloat16)


def _host_prep(x, src, dst, edge_attr, g_W, g_b, f_W, f_b, cls_W):
    """Sort/shard/pack everything the device program needs."""
    order = np.argsort(dst, kind="stable")
    s_src = src[order]
    s_dst = dst[order]
    s_attr = edge_attr[order]

    core_lo = np.searchsorted(s_dst, np.arange(N_CORES) * NPC)
    core_hi = np.searchsorted(s_dst, (np.arange(N_CORES) + 1) * NPC)

    n0 = np.zeros((N_CORES, WPC), np.int64)
    n1 = np.zeros((N_CORES, WPC), np.int64)
    segs = []
    for c in range(N_CORES):
        sl = slice(core_lo[c], core_hi[c])
        ld = s_dst[sl] - c * NPC
        wb = np.searchsorted(ld, np.arange(WPC + 1) * 128)
        b0 = s_src[sl] < BANK
        segs.append((sl, ld, wb, b0))
        for w in range(WPC):
            seg = slice(wb[w], wb[w + 1])
            n0[c, w] = int(b0[seg].sum())
            n1[c, w] = int(seg.stop - seg.start) - n0[c, w]

    # SPMD-identical tile counts: max over cores, rounded up to whole tiles
    T0 = np.maximum(1, -(-n0.max(axis=0) // 128)).astype(np.int64)
    T1 = np.maximum(1, -(-n1.max(axis=0) // 128)).astype(np.int64)

    # Window groups: one dma_gather call per (group, bank); cap per-call tiles
    groups = []  # list of lists of window ids
    cur, a0, a1 = [], 0, 0
    for w in range(WPC):
        if cur and (a0 + T0[w] > TCAP or a1 + T1[w] > TCAP):
            groups.append(cur)
            cur, a0, a1 = [], 0, 0
        cur.append(w)
        a0 += int(T0[w])
        a1 += int(T1[w])
    if cur:
        groups.append(cur)

    # Slot layout: per group: [bank0 tiles of all windows | bank1 tiles]
    # gmeta: per group: (gtile0, Tg, [(w, off0, t0, off1, t1), ...], b0_tiles)
    gmeta = []
    gtile = 0
    for ws in groups:
        b0_tiles = int(sum(int(T0[w]) for w in ws))
        b1_tiles = int(sum(int(T1[w]) for w in ws))
        wins = []
        o0, o1 = 0, b0_tiles
        for w in ws:
            wins.append((w, o0, int(T0[w]), o1, int(T1[w])))
            o0 += int(T0[w])
            o1 += int(T1[w])
        gmeta.append((gtile, b0_tiles + b1_tiles, wins, b0_tiles))
        gtile += b0_tiles + b1_tiles
    ntiles = gtile
    nslot = ntiles * 128

    per_core = []
    for c in range(N_CORES):
        sl, ld, wb, b0 = segs[c]
        idx_arr = np.zeros(nslot, np.int16)
        dstrel = np.full(nslot, -1.0, np.float32)
        attr_arr = np.zeros((nslot, D), np.float32)
        csrc = s_src[sl]
        cattr = s_attr[sl]
        for (gtile0, Tg, wins, b0t) in gmeta:
            for (w, off0, t0, off1, t1) in wins:
                seg = slice(wb[w], wb[w + 1])
                m0 = b0[seg]
                wsrc = csrc[seg]
                wld = ld[seg]
                wat = cattr[seg]
                for bsel, off in ((m0, (gtile0 + off0) * 128),
                                  (~m0, (gtile0 + off1) * 128)):
                    ssrc = wsrc[bsel]
                    o = np.argsort(ssrc, kind="stable")  # src-sorted: DRAM locality
                    k = int(bsel.sum())
                    ssrc = ssrc[o]
                    if bsel is not m0:
                        ssrc = ssrc - BANK
                    idx_arr[off:off + k] = ssrc.astype(np.int16)
                    dstrel[off:off + k] = (wld[bsel][o] - 128 * w).astype(np.float32)
                    attr_arr[off:off + k] = wat[bsel][o]

        idxs_p = np.tile(idx_arr.reshape(nslot // 16, 16).T, (8, 1)).copy()
        attr_p = _bf16(attr_arr.reshape(ntiles, 128, D).transpose(1, 0, 2))
        dstrel_p = np.ascontiguousarray(dstrel.reshape(ntiles, 128).T)

        cnt = np.bincount(ld, minlength=NPC).astype(np.float32)
        inv = 1.0 / np.maximum(cnt, 1.0)
        r = (cnt > 0).astype(np.float32)
        inv_pad = np.ones(NPAD, np.float32)
        inv_pad[:NPC] = inv
        inv_p = inv_pad.reshape(WPC, 128).T.copy()

        x_loc = x[c * NPC : (c + 1) * NPC]
        xTr = np.zeros((C + 1, NPAD), np.float32)
        xTr[:C, :NPC] = (x_loc * r[:, None]).T
        xTr[C, :NPC] = r
        xT1 = np.zeros((C + 1, NPAD), np.float32)
        xT1[:C, :NPC] = x_loc.T
        xT1[C, :NPC] = 1.0

        per_core.append(
            dict(idxs=idxs_p, attr=attr_p, dstrel=dstrel_p, inv=inv_p,
                 xTr=_bf16(xTr), xT1=_bf16(xT1))
        )

    xpad = np.zeros((N_NODES, GROW), np.float32)
    xpad[:, :C] = x
    iota = np.ascontiguousarray(np.broadcast_to(np.arange(128, dtype=np.float32), (128, 128)))
    shared = dict(
        xb0=_bf16(xpad[:BANK]),
        xb1=_bf16(xpad[BANK:]),
        ident=_bf16(np.eye(128, dtype=np.float32)),
        iota=iota,
        Wg1b=_bf16(np.concatenate([g_W[:C], g_b[None]], 0)),
        Wg23=_bf16(g_W[C:]),
        Wf1b=_bf16(np.concatenate([f_W[:C], f_b[None]], 0)),
        Wf2=_bf16(f_W[C:]),
        Wcls=_bf16(cls_W),
    )
    return per_core, shared, gmeta, ntiles


def _build(gmeta, ntiles, cls_b):
    from concourse import bacc, tile, library_config
    from concourse import mybir

    f32 = mybir.dt.float32
    bf16 = mybir.dt.bfloat16
    fp8 = mybir.dt.float8e4
    nslot = ntiles * 128
    nc = bacc.Bacc(None, num_swdge_queues=4, dynamic_dma_scratch_size=32768)

    xb0_d = nc.declare_dram_parameter("xb0", [BANK, GROW], bf16, isOutput=False)
    xb1_d = nc.declare_dram_parameter("xb1", [N_NODES - BANK, GROW], bf16, isOutput=False)
    idxs_d = nc.declare_dram_parameter("idxs", [128, nslot // 16], mybir.dt.int16, isOutput=False)
    attr_d = nc.declare_dram_parameter("attr", [128, ntiles, D], bf16, isOutput=False)
    dstrel_d = nc.declare_dram_parameter("dstrel", [128, ntiles], f32, isOutput=False)
    ident_d = nc.declare_dram_parameter("ident", [128, 128], bf16, isOutput=False)
    iota_d = nc.declare_dram_parameter("iota", [128, 128], f32, isOutput=False)
    inv_d = nc.declare_dram_parameter("inv", [128, WPC], f32, isOutput=False)
    xTr_d = nc.declare_dram_parameter("xTr", [C + 1, NPAD], bf16, isOutput=False)
    xT1_d = nc.declare_dram_parameter("xT1", [C + 1, NPAD], bf16, isOutput=False)
    Wg1b_d = nc.declare_dram_parameter("Wg1b", [C + 1, C], bf16, isOutput=False)
    Wg23_d = nc.declare_dram_parameter("Wg23", [C + D, C], bf16, isOutput=False)
    Wf1b_d = nc.declare_dram_parameter("Wf1b", [C + 1, C], bf16, isOutput=False)
    Wf2_d = nc.declare_dram_parameter("Wf2", [C, C], bf16, isOutput=False)
    Wcls_d = nc.declare_dram_parameter("Wcls", [C, 1], bf16, isOutput=False)
    out_d = nc.declare_dram_parameter("out", [NPC], f32, isOutput=True)

    TGMAX = max(Tg for (_, Tg, _, _) in gmeta)
    glim = int(os.environ.get("KERNEL_GLIM", len(gmeta)))

    with tile.TileContext(nc) as tc:
        with (
            tc.tile_pool(name="const", bufs=1) as constp,
            tc.tile_pool(name="gx", bufs=2) as gxp,
            tc.tile_pool(name="attr", bufs=2) as attrp,
            tc.tile_pool(name="ix", bufs=2) as ixp,
            tc.tile_pool(name="m", bufs=2) as mp,
            tc.tile_pool(name="nodesb", bufs=2) as nsb,
            tc.tile_pool(name="pacc", bufs=2, space="PSUM") as pacc,
            tc.tile_pool(name="pt", bufs=1, space="PSUM") as ptp,
            tc.tile_pool(name="pn", bufs=1, space="PSUM") as pnp,
        ):
            nc.gpsimd.load_library(library_config.mlp)

            ident = constp.tile([128, 128], bf16)
            iota = constp.tile([128, 128], f32)
            dstrel = constp.tile([128, ntiles], f32)
            inv = constp.tile([128, WPC], f32)
            xTr = constp.tile([C + 1, NPAD], bf16)
            xT1 = constp.tile([C + 1, NPAD], bf16)
            Wg1b = constp.tile([C + 1, C], bf16)
            Wg23 = constp.tile([C + D, C], bf16)
            Wf1b = constp.tile([C + 1, C], bf16)
            Wf2 = constp.tile([C, C], bf16)
            Wcls = constp.tile([C, 1], bf16)
            probs = constp.tile([1, NPAD], f32)

            nc.sync.dma_start(ident[:], ident_d[:])
            nc.sync.dma_start(iota[:], iota_d[:])
            nc.sync.dma_start(dstrel[:], dstrel_d[:])
            nc.sync.dma_start(inv[:], inv_d[:])
            nc.sync.dma_start(xTr[:], xTr_d[:])
            nc.sync.dma_start(xT1[:], xT1_d[:])
            nc.sync.dma_start(Wg1b[:], Wg1b_d[:])
            nc.sync.dma_start(Wg23[:], Wg23_d[:])
            nc.sync.dma_start(Wf1b[:], Wf1b_d[:])
            nc.sync.dma_start(Wf2[:], Wf2_d[:])
            nc.sync.dma_start(Wcls[:], Wcls_d[:])

            qrr = [0]
            for (gtile0, Tg, wins, b0t) in gmeta[:glim]:
                gx = gxp.tile([128, TGMAX, GROW], bf16, tag="gx")
                at = attrp.tile([128, TGMAX, D], bf16, tag="attr")
                ix = ixp.tile([128, TGMAX * 8], mybir.dt.int16, tag="ix")
                nc.sync.dma_start(at[:, 0:Tg, :], attr_d[:, gtile0:gtile0 + Tg, :])
                nc.sync.dma_start(ix[:, 0:Tg * 8],
                                  idxs_d[:, gtile0 * 8:(gtile0 + Tg) * 8])

                # one gather call per (group, bank)
                for xb, lo, n in ((xb0_d, 0, b0t * 128),
                                  (xb1_d, b0t * 128, (Tg - b0t) * 128)):
                    nc.gpsimd.dma_gather(
                        gx[:, lo // 128 : (lo + n) // 128, :],
                        xb[:],
                        ix[:, lo // 16 : (lo + n) // 16],
                        n, n, GROW, elem_step=GROW,
                        single_packet=False,
                        queue_num=qrr[0] % 4,
                    )
                    qrr[0] += 1

                # edge_attr into the padding columns of the gathered tiles
                nc.scalar.activation(gx[:, 0:Tg, C : C + D], at[:, 0:Tg, :],
                                     mybir.ActivationFunctionType.Copy)

                # one-hot M for ALL of this group's tiles in one DVE op:
                # M[e, t, j] = (dstrel[e, gtile0+t] == j)
                mg = mp.tile([128, TGMAX, 128], fp8, tag="m")
                nc.vector.tensor_tensor(
                    mg[:, 0:Tg, :],
                    dstrel[:, gtile0:gtile0 + Tg].unsqueeze(-1)
                        .broadcast_to([128, Tg, 128]),
                    iota[:, 0:128].unsqueeze(1).broadcast_to([128, Tg, 128]),
                    op=mybir.AluOpType.is_equal)

                for (w, off0, t0, off1, t1) in wins:
                    acc = pacc.tile([128, RHS], f32, tag="acc")
                    tiles = [off0 + i for i in range(t0)] + \
                            [off1 + i for i in range(t1)]
                    for k, t in enumerate(tiles):
                        nc.tensor.matmul(acc[:], mg[:, t, :], gx[:, t, 0:RHS],
                                         start=(k == 0), stop=(k == len(tiles) - 1))

                    # PSUM -> SBUF with 1/cnt scaling fused into the ACT copy
                    s = nsb.tile([128, RHS], bf16, tag="s")
                    nc.scalar.activation(s[:], acc[:],
                                         mybir.ActivationFunctionType.Copy,
                                         scale=inv[:, w : w + 1])

                    # transpose to feature-major
                    pt1 = ptp.tile([RHS, 128], bf16, tag="pt1")
                    nc.tensor.transpose(pt1[:], s[:], ident[:])
                    sT = nsb.tile([RHS, 128], bf16, tag="sT")
                    nc.scalar.activation(sT[:], pt1[:],
                                         mybir.ActivationFunctionType.Copy)

                    cols = slice(w * 128, (w + 1) * 128)
                    pag = pnp.tile([C, 128], f32, tag="pag")
                    nc.tensor.matmul(pag[:], Wg1b[:], xTr[:, cols], start=True, stop=False)
                    nc.tensor.matmul(pag[:], Wg23[:], sT[:], start=False, stop=True)
                    aggrT = nsb.tile([C, 128], bf16, tag="aggrT")
                    nc.scalar.activation(aggrT[:], pag[:],
                                         mybir.ActivationFunctionType.Copy)

                    ph = pnp.tile([C, 128], f32, tag="ph")
                    nc.tensor.matmul(ph[:], Wf1b[:], xT1[:, cols], start=True, stop=False)
                    nc.tensor.matmul(ph[:], Wf2[:], aggrT[:], start=False, stop=True)
                    hT = nsb.tile([C, 128], bf16, tag="hT")
                    nc.scalar.activation(hT[:], ph[:],
                                         mybir.ActivationFunctionType.Copy)

                    pl = pnp.tile([1, 128], f32, tag="pl")
                    nc.tensor.matmul(pl[:], Wcls[:], hT[:], start=True, stop=True)
                    nc.scalar.activation(probs[0:1, cols], pl[:],
                                         mybir.ActivationFunctionType.Sigmoid,
                                         bias=float(cls_b))

            nc.sync.dma_start(out_d[0:NPC], probs[0:1, 0:NPC])

    nc.compile()
    return nc


def _exact_patch(probs, sel, x, src, dst, edge_attr, g_W, g_b, f_W, f_b,
                 cls_W, cls_b):
    """Recompute probs exactly (f64) for the selected nodes."""
    if not sel.any():
        return probs
    nodes = np.nonzero(sel)[0]
    order = np.argsort(dst, kind="stable")
    s_src = src[order]
    s_dst = dst[order]
    s_attr = edge_attr[order].astype(np.float64)
    lo = np.searchsorted(s_dst, nodes)
    hi = np.searchsorted(s_dst, nodes + 1)
    x64 = x.astype(np.float64)
    gW = g_W.astype(np.float64)
    fW = f_W.astype(np.float64)
    for i, node in enumerate(nodes):
        e = slice(lo[i], hi[i])
        cntv = hi[i] - lo[i]
        if cntv > 0:
            z = np.concatenate([
                np.broadcast_to(x64[node], (cntv, C)),
                x64[s_src[e]],
                s_attr[e],
            ], axis=1)
            aggr = (z @ gW + g_b).sum(0) / cntv
        else:
            aggr = np.zeros(C)
        h = np.concatenate([x64[node], aggr]) @ fW + f_b
        logit = h @ cls_W.astype(np.float64)[:, 0] + cls_b
        probs[node] = 1.0 / (1.0 + np.exp(-logit))
    return probs


def kernel(x, edge_index, edge_attr, g_W, g_b, f_W, f_b, cls_W, cls_b):
    from concourse.bass_utils import run_bass_kernel_spmd

    x = np.asarray(x, np.float32)
    edge_attr = np.asarray(edge_attr, np.float32)
    src = np.asarray(edge_index[0], np.int64)
    dst = np.asarray(edge_index[1], np.int64)
    g_W = np.asarray(g_W, np.float32)
    g_b = np.asarray(g_b, np.float32)
    f_W = np.asarray(f_W, np.float32)
    f_b = np.asarray(f_b, np.float32)
    cls_W = np.asarray(cls_W, np.float32)
    cls_bv = float(np.asarray(cls_b).reshape(-1)[0])

    per_core, shared, gmeta, ntiles = _host_prep(
        x, src, dst, edge_attr, g_W, g_b, f_W, f_b, cls_W)
    nc = _build(gmeta, ntiles, cls_bv)

    in_maps = [{**shared, **pc} for pc in per_core]
    res = run_bass_kernel_spmd(nc, in_maps, core_ids=list(range(N_CORES)))
    probs = np.concatenate([res.results[c]["out"] for c in range(N_CORES)])
    probs = probs.astype(np.float64)

    # bf16 device math can flip the 0.5 threshold for near-boundary nodes;
    # recompute those exactly.
    sel = np.abs(probs - 0.5) < 2e-3
    probs = _exact_patch(probs, sel, x, src, dst, edge_attr, g_W, g_b,
                         f_W, f_b, cls_W, cls_bv)

    probs = probs.reshape(N_NODES, 1).astype(np.float32)
    y = probs > 0.5
    return (y, probs)


# revision 11
# speedup vs baseline: 1.2839x; 1.1393x over previous
"""CutsSelector GNN message-passing kernel for 8 Trainium2 NeuronCores.

Strategy (destination-sharded, no collectives):
  - Edges are sorted by dst on the host; core c owns nodes [c*6250, (c+1)*6250)
    and all edges pointing into that range.
  - Linearity of the message MLP:
        segsum(concat(x_dst, x_src, e) @ g_W + g_b, dst)
      = cnt*(x@gW1 + g_b) + segsum(x[src])@gW2 + segsum(e)@gW3
    so only the raw per-edge features (x[src] rows, edge_attr rows) need the
    segmented reduce; all matmuls happen at node granularity.
  - x[src] rows are fetched with the SWDGE indexed gather (dma_gather) from
    bf16 x banks padded to 256B rows.  int16 index limit -> two 25000-row
    banks.  Windows are batched into GROUPS so each (group, bank) is ONE
    dma_gather call (<= 16368 idxs, the Q7 int32-scratch limit); this cuts
    per-call GPSIMD descriptor-gen overhead ~7x vs per-window calls.
  - Within each (window, bank) edges are sorted by src so the gathered rows
    hit DRAM in ascending address order (page locality).
  - Segmented reduce: per 128-edge tile a one-hot M[e, j] = (dst_rel[e] == j)
    is built ON DEVICE by the idle DVE (tensor_scalar is_equal of a constant
    iota row against the per-slot dst_rel), output fp8; ONE TensorE matmul
    per tile accumulates [segsum_x | segsum_attr] into PSUM.  This removes
    the 26MB host-baked one-hot DMA of the previous version.
  - Node phase per window: 1/cnt scaling fused into the ACT PSUM->SBUF copy,
    PE transpose to feature-major, then g/f/cls MLPs (bf16 in, f32 psum).
  - bf16 rounding can flip the y = probs > 0.5 threshold for nodes whose
    probs sit within ~1e-3 of 0.5; those few nodes are recomputed exactly on
    the host afterwards.
"""
import os
import sys

sys.path.insert(0, "/opt/trn_rl_repo")
os.environ.setdefault("BASS_PERFETTO_PROFILE_ALL_CORES", "1")

import numpy as np

N_NODES = 50000
N_EDGES = 1_600_000
C = 64
D = 16
N_CORES = 8
NPC = N_NODES // N_CORES            # 6250 nodes per core
WPC = (NPC + 127) // 128            # 49 windows per core
NPAD = WPC * 128                    # 6272 padded nodes per core
BANK = 25000                        # gather-bank split (int16 index limit)
GROW = 128                          # gather source row length (bf16 -> 256B)
RHS = C + D                         # 80 columns fed to the reduce matmul
TCAP = 32                           # max tiles per (group, bank) gather call


def _bf16(a):
    import ml_dtypes
    return np.asarray(a, np.float32).astype(ml_dtypes.bfloat16)


def _host_prep(x, src, dst, edge_attr, g_W, g_b, f_W, f_b, cls_W):
    """Sort/shard/pack everything the device program needs."""
    order = np.argsort(dst, kind="stable")
    s_src = src[order]
    s_dst = dst[order]
    s_attr = edge_attr[order]

    core_lo = np.searchsorted(s_dst, np.arange(N_CORES) * NPC)
    core_hi = np.searchsorted(s_dst, (np.arange(N_CORES) + 1) * NPC)

    n0 = np.zeros((N_CORES, WPC), np.int64)
    n1 = np.zeros((N_CORES, WPC), np.int64)
    segs = []
    for c in range(N_CORES):
        sl = slice(core_lo[c], core_hi[c])
        ld = s_dst[sl] - c * NPC
        wb = np.searchsorted(ld, np.arange(WPC + 1) * 128)
        b0 = s_src[sl] < BANK
        segs.append((sl, ld, wb, b0))
        for w in range(WPC):
            seg = slice(wb[w], wb[w + 1])
            n0[c, w] = int(b0[seg].sum())
            n1[c, w] = int(seg.stop - seg.start) - n0[c, w]

    # SPMD-identical tile counts: max over cores, rounded up to whole tiles
    T0 = np.maximum(1, -(-n0.max(axis=0) // 128)).astype(np.int64)
    T1 = np.maximum(1, -(-n1.max(axis=0) // 128)).astype(np.int64)

    # Window groups: one dma_gather call per (group, bank); cap per-call tiles
    groups = []  # list of lists of window ids
    cur, a0, a1 = [], 0, 0
    for w in range(WPC):
        if cur and (a0 + T0[w] > TCAP or a1 + T1[w] > TCAP):
            groups.append(cur)
            cur, a0, a1 = [], 0, 0
        cur.append(w)
        a0 += int(T0[w])
        a1 += int(T1[w])
    if cur:
        groups.append(cur)

    # Slot layout: per group: [bank0 tiles of all windows | bank1 tiles]
    # gmeta: per group: (gtile0, Tg, [(w, off0, t0, off1, t1), ...], b0_tiles)
    gmeta = []
    gtile = 0
    for ws in groups:
        b0_tiles = int(sum(int(T0[w]) for w in ws))
        b1_tiles = int(sum(int(T1[w]) for w in ws))
        wins = []
        o0, o1 = 0, b0_tiles
        for w in ws:
            wins.append((w, o0, int(T0[w]), o1, int(T1[w])))
            o0 += int(T0[w])
            o1 += int(T1[w])
        gmeta.append((gtile, b0_tiles + b1_tiles, wins, b0_tiles))
        gtile += b0_tiles + b1_tiles
    ntiles = gtile
    nslot = ntiles * 128

    per_core = []
    for c in range(N_CORES):
        sl, ld, wb, b0 = segs[c]
        idx_arr = np.zeros(nslot, np.int16)
        dstrel = np.full(nslot, -1.0, np.float32)
        attr_arr = np.zeros((nslot, D), np.float32)
        csrc = s_src[sl]
        cattr = s_attr[sl]
        for (gtile0, Tg, wins, b0t) in gmeta:
            for (w, off0, t0, off1, t1) in wins:
                seg = slice(wb[w], wb[w + 1])
                m0 = b0[seg]
                wsrc = csrc[seg]
                wld = ld[seg]
                wat = cattr[seg]
                for bsel, off in ((m0, (gtile0 + off0) * 128),
                                  (~m0, (gtile0 + off1) * 128)):
                    ssrc = wsrc[bsel]
                    o = np.argsort(ssrc, kind="stable")  # src-sorted: DRAM locality
                    k = int(bsel.sum())
                    ssrc = ssrc[o]
                    if bsel is not m0:
                        ssrc = ssrc - BANK
                    idx_arr[off:off + k] = ssrc.astype(np.int16)
                    dstrel[off:off + k] = (wld[bsel][o] - 128 * w).astype(np.float32)
                    attr_arr[off:off + k] = wat[bsel][o]

        idxs_p = np.tile(idx_arr.reshape(nslot // 16, 16).T, (8, 1)).copy()
        attr_p = _bf16(attr_arr.reshape(ntiles, 128, D).transpose(1, 0, 2))
        dstrel_p = np.ascontiguousarray(dstrel.reshape(ntiles, 128).T)

        cnt = np.bincount(ld, minlength=NPC).astype(np.float32)
        inv = 1.0 / np.maximum(cnt, 1.0)
        r = (cnt > 0).astype(np.float32)
        inv_pad = np.ones(NPAD, np.float32)
        inv_pad[:NPC] = inv
        inv_p = inv_pad.reshape(WPC, 128).T.copy()

        x_loc = x[c * NPC : (c + 1) * NPC]
        xTr = np.zeros((C + 1, NPAD), np.float32)
        xTr[:C, :NPC] = (x_loc * r[:, None]).T
        xTr[C, :NPC] = r
        xT1 = np.zeros((C + 1, NPAD), np.float32)
        xT1[:C, :NPC] = x_loc.T
        xT1[C, :NPC] = 1.0

        per_core.append(
            dict(idxs=idxs_p, attr=attr_p, dstrel=dstrel_p, inv=inv_p,
                 xTr=_bf16(xTr), xT1=_bf16(xT1))
        )

    xpad = np.zeros((N_NODES, GROW), np.float32)
    xpad[:, :C] = x
    iota = np.ascontiguousarray(np.broadcast_to(np.arange(128, dtype=np.float32), (128, 128)))
    shared = dict(
        xb0=_bf16(xpad[:BANK]),
        xb1=_bf16(xpad[BANK:]),
        ident=_bf16(np.eye(128, dtype=np.float32)),
        iota=iota,
        Wg1b=_bf16(np.concatenate([g_W[:C], g_b[None]], 0)),
        Wg23=_bf16(g_W[C:]),
        Wf1b=_bf16(np.concatenate([f_W[:C], f_b[None]], 0)),
        Wf2=_bf16(f_W[C:]),
        Wcls=_bf16(cls_W),
    )
    return per_core, shared, gmeta, ntiles


def _build(gmeta, ntiles, cls_b):
    from concourse import bacc, tile, library_config
    from concourse import mybir

    f32 = mybir.dt.float32
    bf16 = mybir.dt.bfloat16
    fp8 = mybir.dt.float8e4
    nslot = ntiles * 128
    nc = bacc.Bacc(None, num_swdge_queues=4, dynamic_dma_scratch_size=32768)

    xb0_d = nc.declare_dram_parameter("xb0", [BANK, GROW], bf16, isOutput=False)
    xb1_d = nc.declare_dram_parameter("xb1", [N_NODES - BANK, GROW], bf16, isOutput=False)
    idxs_d = nc.declare_dram_parameter("idxs", [128, nslot // 16], mybir.dt.int16, isOutput=False)
    attr_d = nc.declare_dram_parameter("attr", [128, ntiles, D], bf16, isOutput=False)
    dstrel_d = nc.declare_dram_parameter("dstrel", [128, ntiles], f32, isOutput=False)
    ident_d = nc.declare_dram_parameter("ident", [128, 128], bf16, isOutput=False)
    iota_d = nc.declare_dram_parameter("iota", [128, 128], f32, isOutput=False)
    inv_d = nc.declare_dram_parameter("inv", [128, WPC], f32, isOutput=False)
    xTr_d = nc.declare_dram_parameter("xTr", [C + 1, NPAD], bf16, isOutput=False)
    xT1_d = nc.declare_dram_parameter("xT1", [C + 1, NPAD], bf16, isOutput=False)
    Wg1b_d = nc.declare_dram_parameter("Wg1b", [C + 1, C], bf16, isOutput=False)
    Wg23_d = nc.declare_dram_parameter("Wg23", [C + D, C], bf16, isOutput=False)
    Wf1b_d = nc.declare_dram_parameter("Wf1b", [C + 1, C], bf16, isOutput=False)
    Wf2_d = nc.declare_dram_parameter("Wf2", [C, C], bf16, isOutput=False)
    Wcls_d = nc.declare_dram_parameter("Wcls", [C, 1], bf16, isOutput=False)
    out_d = nc.declare_dram_parameter("out", [NPC], f32, isOutput=True)

    TGMAX = max(Tg for (_, Tg, _, _) in gmeta)
    glim = int(os.environ.get("KERNEL_GLIM", len(gmeta)))

    with tile.TileContext(nc) as tc:
        with (
            tc.tile_pool(name="const", bufs=1) as constp,
            tc.tile_pool(name="gx", bufs=2) as gxp,
            tc.tile_pool(name="attr", bufs=2) as attrp,
            tc.tile_pool(name="ix", bufs=2) as ixp,
            tc.tile_pool(name="m", bufs=2) as mp,
            tc.tile_pool(name="nodesb", bufs=2) as nsb,
            tc.tile_pool(name="pacc", bufs=2, space="PSUM") as pacc,
            tc.tile_pool(name="pt", bufs=1, space="PSUM") as ptp,
            tc.tile_pool(name="pn", bufs=1, space="PSUM") as pnp,
        ):
            nc.gpsimd.load_library(library_config.mlp)

            ident = constp.tile([128, 128], bf16)
            iota = constp.tile([128, 128], f32)
            dstrel = constp.tile([128, ntiles], f32)
            inv = constp.tile([128, WPC], f32)
            xTr = constp.tile([C + 1, NPAD], bf16)
            xT1 = constp.tile([C + 1, NPAD], bf16)
            Wg1b = constp.tile([C + 1, C], bf16)
            Wg23 = constp.tile([C + D, C], bf16)
            Wf1b = constp.tile([C + 1, C], bf16)
            Wf2 = constp.tile([C, C], bf16)
            Wcls = constp.tile([C, 1], bf16)
            probs = constp.tile([1, NPAD], f32)

            nc.sync.dma_start(ident[:], ident_d[:])
            nc.sync.dma_start(iota[:], iota_d[:])
            nc.sync.dma_start(dstrel[:], dstrel_d[:])
            nc.sync.dma_start(inv[:], inv_d[:])
            nc.sync.dma_start(xTr[:], xTr_d[:])
            nc.sync.dma_start(xT1[:], xT1_d[:])
            nc.sync.dma_start(Wg1b[:], Wg1b_d[:])
            nc.sync.dma_start(Wg23[:], Wg23_d[:])
            nc.sync.dma_start(Wf1b[:], Wf1b_d[:])
            nc.sync.dma_start(Wf2[:], Wf2_d[:])
            nc.sync.dma_start(Wcls[:], Wcls_d[:])

            qrr = [0]
            for (gtile0, Tg, wins, b0t) in gmeta[:glim]:
                gx = gxp.tile([128, TGMAX, GROW], bf16, tag="gx")
                at = attrp.tile([128, TGMAX, D], bf16, tag="attr")
                ix = ixp.tile([128, TGMAX * 8], mybir.dt.int16, tag="ix")
                nc.sync.dma_start(at[:, 0:Tg, :], attr_d[:, gtile0:gtile0 + Tg, :])
                nc.sync.dma_start(ix[:, 0:Tg * 8],
                                  idxs_d[:, gtile0 * 8:(gtile0 + Tg) * 8])

                # one gather call per (group, bank)
                for xb, lo, n in ((xb0_d, 0, b0t * 128),
                                  (xb1_d, b0t * 128, (Tg - b0t) * 128)):
                    nc.gpsimd.dma_gather(
                        gx[:, lo // 128 : (lo + n) // 128, :],
                        xb[:],
                        ix[:, lo // 16 : (lo + n) // 16],
                        n, n, GROW, elem_step=GROW,
                        single_packet=False,
                        queue_num=qrr[0] % 4,
                    )
                    qrr[0] += 1

                # edge_attr into the padding columns of the gathered tiles
                nc.scalar.activation(gx[:, 0:Tg, C : C + D], at[:, 0:Tg, :],
                                     mybir.ActivationFunctionType.Copy)

                # one-hot M for ALL of this group's tiles in one DVE op:
                # M[e, t, j] = (dstrel[e, gtile0+t] == j)
                mg = mp.tile([128, TGMAX, 128], fp8, tag="m")
                nc.vector.tensor_tensor(
                    mg[:, 0:Tg, :],
                    dstrel[:, gtile0:gtile0 + Tg].unsqueeze(-1)
                        .broadcast_to([128, Tg, 128]),
                    iota[:, 0:128].unsqueeze(1).broadcast_to([128, Tg, 128]),
                    op=mybir.AluOpType.is_equal)

                for (w, off0, t0, off1, t1) in wins:
                    acc = pacc.tile([128, RHS], f32, tag="acc")
                    tiles = [off0 + i for i in range(t0)] + \
                            [off1 + i for i in range(t1)]
                    for k, t in enumerate(tiles):
                        nc.tensor.matmul(acc[:], mg[:, t, :], gx[:, t, 0:RHS],
                                         start=(k == 0), stop=(k == len(tiles) - 1))

                    # PSUM -> SBUF with 1/cnt scaling fused into the ACT copy
                    s = nsb.tile([128, RHS], bf16, tag="s")
                    nc.scalar.activation(s[:], acc[:],
                                         mybir.ActivationFunctionType.Copy,
                                         scale=inv[:, w : w + 1])

                    # transpose to feature-major
                    pt1 = ptp.tile([RHS, 128], bf16, tag="pt1")
                    nc.tensor.transpose(pt1[:], s[:], ident[:])
                    sT = nsb.tile([RHS, 128], bf16, tag="sT")
                    nc.scalar.activation(sT[:], pt1[:],
                                         mybir.ActivationFunctionType.Copy)

                    cols = slice(w * 128, (w + 1) * 128)
                    pag = pnp.tile([C, 128], f32, tag="pag")
                    nc.tensor.matmul(pag[:], Wg1b[:], xTr[:, cols], start=True, stop=False)
                    nc.tensor.matmul(pag[:], Wg23[:], sT[:], start=False, stop=True)
                    aggrT = nsb.tile([C, 128], bf16, tag="aggrT")
                    nc.scalar.activation(aggrT[:], pag[:],
                                         mybir.ActivationFunctionType.Copy)

                    ph = pnp.tile([C, 128], f32, tag="ph")
                    nc.tensor.matmul(ph[:], Wf1b[:], xT1[:, cols], start=True, stop=False)
                    nc.tensor.matmul(ph[:], Wf2[:], aggrT[:], start=False, stop=True)
                    hT = nsb.tile([C, 128], bf16, tag="hT")
                    nc.scalar.activation(hT[:], ph[:],
                                         mybir.ActivationFunctionType.Copy)

                    pl = pnp.tile([1, 128], f32, tag="pl")
                    nc.tensor.matmul(pl[:], Wcls[:], hT[:], start=True, stop=True)
                    nc.scalar.activation(probs[0:1, cols], pl[:],
                                         mybir.ActivationFunctionType.Sigmoid,
                                         bias=float(cls_b))

            nc.sync.dma_start(out_d[0:NPC], probs[0:1, 0:NPC])

    nc.compile()
    return nc


def _exact_patch(probs, sel, x, src, dst, edge_attr, g_W, g_b, f_W, f_b,
                 cls_W, cls_b):
    """Recompute probs exactly (f64) for the selected nodes."""
    if not sel.any():
        return probs
    nodes = np.nonzero(sel)[0]
    order = np.argsort(dst, kind="stable")
    s_src = src[order]
    s_dst = dst[order]
    s_attr = edge_attr[order].astype(np.float64)
    lo = np.searchsorted(s_dst, nodes)
    hi = np.searchsorted(s_dst, nodes + 1)
    x64 = x.astype(np.float64)
    gW = g_W.astype(np.float64)
    fW = f_W.astype(np.float64)
    for i, node in enumerate(nodes):
        e = slice(lo[i], hi[i])
        cntv = hi[i] - lo[i]
        if cntv > 0:
            z = np.concatenate([
                np.broadcast_to(x64[node], (cntv, C)),
                x64[s_src[e]],
                s_attr[e],
            ], axis=1)
            aggr = (z @ gW + g_b).sum(0) / cntv
        else:
            aggr = np.zeros(C)
        h = np.concatenate([x64[node], aggr]) @ fW + f_b
        logit = h @ cls_W.astype(np.float64)[:, 0] + cls_b
        probs[node] = 1.0 / (1.0 + np.exp(-logit))
    return probs


def kernel(x, edge_index, edge_attr, g_W, g_b, f_W, f_b, cls_W, cls_b):
    from concourse.bass_utils import run_bass_kernel_spmd

    x = np.asarray(x, np.float32)
    edge_attr = np.asarray(edge_attr, np.float32)
    src = np.asarray(edge_index[0], np.int64)
    dst = np.asarray(edge_index[1], np.int64)
    g_W = np.asarray(g_W, np.float32)
    g_b = np.asarray(g_b, np.float32)
    f_W = np.asarray(f_W, np.float32)
    f_b = np.asarray(f_b, np.float32)
    cls_W = np.asarray(cls_W, np.float32)
    cls_bv = float(np.asarray(cls_b).reshape(-1)[0])

    per_core, shared, gmeta, ntiles = _host_prep(
        x, src, dst, edge_attr, g_W, g_b, f_W, f_b, cls_W)
    nc = _build(gmeta, ntiles, cls_bv)

    in_maps = [{**shared, **pc} for pc in per_core]
    res = run_bass_kernel_spmd(nc, in_maps, core_ids=list(range(N_CORES)))
    probs = np.concatenate([res.results[c]["out"] for c in range(N_CORES)])
    probs = probs.astype(np.float64)

    # bf16 device math can flip the 0.5 threshold for near-boundary nodes;
    # recompute those exactly.
    sel = np.abs(probs - 0.5) < 2e-3
    probs = _exact_patch(probs, sel, x, src, dst, edge_attr, g_W, g_b,
                         f_W, f_b, cls_W, cls_bv)

    probs = probs.reshape(N_NODES, 1).astype(np.float32)
    y = probs > 0.5
    return (y, probs)
